# revision 1
# baseline (speedup 1.0000x reference)
# Gaussian-smoothing heatmap kernel for trn2 (8 NeuronCores, data-parallel).
#
# Math: each heatmap channel is a single one-hot spike (or empty), so the
# 24->24 5x5 conv is equivalent to stamping the flipped 5x5 filter at each
# keypoint and summing over input channels.  All (o,i) filter slices are the
# identical binomial gaussian, so every output channel of a batch equals the
# same 64x64 accumulated map M[b].  We compute M[b] on-device as a 120-way
# contraction matmul (rows = (u,k) tap/keypoint pairs) between one-hot row
# selectors and column stamp profiles, flatten each map into one SBUF
# partition, and write out[b, o] for all 24 o with replicating DMAs whose
# contiguous runs are 16KB.
import numpy as np

B_FULL = 1024
K = 24
H = 64
N_CORES = 8
B_LOC = B_FULL // N_CORES  # 128
GB = 32                    # batches per pipeline group
NG = B_LOC // GB           # 4 groups
MAGIC = 12582912.0         # 1.5 * 2^23: RNE integer rounding for |v| < 2^22
SENT = 4096.0              # sentinel shift for masked-out keypoints

_CACHE = {}


def _build_nc():
    import concourse.mybir as mybir
    from concourse import bacc
    from concourse.tile import TileContext

    fp32 = mybir.dt.float32
    i32 = mybir.dt.int32
    Alu = mybir.AluOpType

    nc = bacc.Bacc()
    xin = nc.dram_tensor("xin", [B_LOC, K * 2], fp32, kind="ExternalInput")
    visb = nc.dram_tensor("visb", [256], i32, kind="ExternalInput")
    visk = nc.dram_tensor("visk", [256], i32, kind="ExternalInput")
    boff = nc.dram_tensor("boff", [128, 1], fp32, kind="ExternalInput")
    wg = nc.dram_tensor("wg", [120, 120], fp32, kind="ExternalInput")
    eye = nc.dram_tensor("eye", [128, 128], fp32, kind="ExternalInput")
    outT = nc.dram_tensor("out", [B_LOC, K, H * H], fp32, kind="ExternalOutput")

    with TileContext(nc) as tc:
        with (
            tc.tile_pool(name="const", bufs=1) as cpool,
            tc.tile_pool(name="small", bufs=2) as spool,
            tc.tile_pool(name="big", bufs=2) as bpool,
            tc.tile_pool(name="stage", bufs=2) as stpool,
            tc.tile_pool(name="fbuf", bufs=4) as fpool,
            tc.tile_pool(name="dram", bufs=2, space="DRAM") as dpool,
            tc.tile_pool(name="ps_pre", bufs=1, space="PSUM") as ps_pre,
            tc.tile_pool(name="ps_tr", bufs=2, space="PSUM") as ps_tr,
            tc.tile_pool(name="ps_b", bufs=2, space="PSUM") as ps_b,
            tc.tile_pool(name="ps_map", bufs=3, space="PSUM") as ps_map,
        ):
            # ---------------- inputs ----------------
            xt = cpool.tile([B_LOC, K * 2], fp32)
            nc.sync.dma_start(xt, xin[:, :])
            vbt = cpool.tile([128, 2], i32)
            nc.sync.dma_start(vbt, visb[:].rearrange("(t p) -> p t", p=128))
            vkt = cpool.tile([128, 2], i32)
            nc.sync.dma_start(vkt, visk[:].rearrange("(t p) -> p t", p=128))
            bofft = cpool.tile([128, 1], fp32)
            nc.sync.dma_start(bofft, boff[:, :])
            wgt = cpool.tile([120, 120], fp32)
            nc.sync.dma_start(wgt, wg[:, :])
            eyet = cpool.tile([128, 128], fp32)
            nc.sync.dma_start(eyet, eye[:, :])

            # ---------------- iotas ----------------
            io24i = spool.tile([128, K], i32)
            nc.gpsimd.iota(io24i, pattern=[[1, K]], base=0, channel_multiplier=0)
            io24f = cpool.tile([128, K], fp32)
            nc.vector.tensor_copy(io24f, io24i)
            io128i = spool.tile([128, 128], i32)
            nc.gpsimd.iota(io128i, pattern=[[1, 128]], base=0, channel_multiplier=0)
            io128f = cpool.tile([128, 128], fp32)
            nc.vector.tensor_copy(io128f, io128i)
            io64i = spool.tile([120, H], i32)
            nc.gpsimd.iota(io64i, pattern=[[1, H]], base=0, channel_multiplier=0)
            io64f = cpool.tile([120, H], fp32)
            nc.vector.tensor_copy(io64f, io64i)

            # ---------------- coords: round(((x+1)*0.5)*63), bit-exact RNE ----
            c1 = spool.tile([B_LOC, K * 2], fp32)
            nc.vector.tensor_scalar(c1, xt, 0.5, 0.5, Alu.mult, Alu.add)
            nc.vector.tensor_scalar_mul(c1, c1, 63.0)
            nc.vector.tensor_scalar_add(c1, c1, MAGIC)
            coords = spool.tile([B_LOC, K * 2], fp32)
            nc.vector.tensor_scalar_sub(coords, c1, MAGIC)
            c3 = coords.rearrange("p (k c) -> p k c", c=2)
            cx = c3[:, :, 0]
            cy = c3[:, :, 1]

            # valid = in [0, 63] for both coords
            ge = spool.tile([B_LOC, K * 2], fp32)
            nc.vector.tensor_single_scalar(ge, coords, 0.0, Alu.is_ge)
            le = spool.tile([B_LOC, K * 2], fp32)
            nc.vector.tensor_single_scalar(le, coords, 63.0, Alu.is_le)
            vp = spool.tile([B_LOC, K * 2], fp32)
            nc.vector.tensor_mul(vp, ge, le)
            vp3 = vp.rearrange("p (k c) -> p k c", c=2)
            valid = spool.tile([B_LOC, K], fp32)
            nc.vector.tensor_mul(valid, vp3[:, :, 0], vp3[:, :, 1])

            cxz = spool.tile([B_LOC, K], fp32)
            nc.vector.tensor_mul(cxz, valid, cx)
            place = spool.tile([B_LOC, K], fp32)
            nc.vector.tensor_single_scalar(place, cxz, 0.0, Alu.not_equal)

            # ---------------- kill mask from vis_batch/vis_kps ----------------
            vbf = spool.tile([128, 2], fp32)
            nc.vector.tensor_copy(vbf, vbt)
            vbl = spool.tile([128, 2], fp32)
            nc.vector.tensor_scalar(vbl, vbf, bofft, None, Alu.subtract)
            vkf = spool.tile([128, 2], fp32)
            nc.vector.tensor_copy(vkf, vkt)

            killp = ps_pre.tile([128, K], fp32)
            for t in range(2):
                ohB = spool.tile([128, 128], fp32, tag="ohB")
                nc.vector.tensor_scalar(ohB, io128f, vbl[:, t : t + 1], None, Alu.is_equal)
                ohK = spool.tile([128, K], fp32, tag="ohK")
                nc.vector.tensor_scalar(ohK, io24f, vkf[:, t : t + 1], None, Alu.is_equal)
                nc.tensor.matmul(killp, lhsT=ohB, rhs=ohK, start=(t == 0), stop=(t == 1))
            kill01 = spool.tile([B_LOC, K], fp32)
            nc.vector.tensor_single_scalar(kill01, killp, 0.5, Alu.is_gt)

            # mask = place * (1 - kill)
            pk = spool.tile([B_LOC, K], fp32)
            nc.vector.tensor_mul(pk, place, kill01)
            mask = spool.tile([B_LOC, K], fp32)
            nc.vector.tensor_sub(mask, place, pk)

            # cyS = cy - SENT*mask   (later add (u + SENT - 2) per tap row)
            cyS = spool.tile([B_LOC, K], fp32)
            nc.vector.scalar_tensor_tensor(cyS, mask, -SENT, cy, Alu.mult, Alu.add)

            # PYpack[b, u*24+k] = cyS + u + SENT - 2 ; PXpack[b, c*24+k] = cxz + c - 2
            pypack = spool.tile([B_LOC, 120], fp32, tag="pypack")
            pxpack = spool.tile([B_LOC, 120], fp32, tag="pxpack")
            for u in range(5):
                nc.vector.tensor_scalar_add(pypack[:, u * K : (u + 1) * K], cyS, float(u) + SENT - 2.0)
                nc.vector.tensor_scalar_add(pxpack[:, u * K : (u + 1) * K], cxz, float(u) - 2.0)

            # transpose to [(tap,k)=120, b=128]
            pyt_ps = ps_tr.tile([120, 128], fp32, tag="tr")
            nc.tensor.transpose(pyt_ps, pypack, eyet)
            PYT = cpool.tile([120, 128], fp32)
            nc.vector.tensor_copy(PYT, pyt_ps)
            pxt_ps = ps_tr.tile([120, 128], fp32, tag="tr")
            nc.tensor.transpose(pxt_ps, pxpack, eyet)
            PXT = cpool.tile([120, 128], fp32)
            nc.vector.tensor_copy(PXT, pxt_ps)

            # ---------------- main pipeline, groups of GB batches ------------
            order = list(range(NG))
            for g in order:
                b0 = g * GB
                rowsel = bpool.tile([120, GB * H], fp32, tag="rowsel")
                nc.vector.tensor_tensor(
                    rowsel.rearrange("p (b y) -> p b y", y=H),
                    io64f.unsqueeze(1).broadcast_to([120, GB, H]),
                    PYT[:, b0 : b0 + GB].unsqueeze(2).broadcast_to([120, GB, H]),
                    Alu.is_equal,
                )
                ohc = bpool.tile([120, GB * H], fp32, tag="ohc")
                nc.vector.tensor_tensor(
                    ohc.rearrange("p (b x) -> p b x", x=H),
                    io64f.unsqueeze(1).broadcast_to([120, GB, H]),
                    PXT[:, b0 : b0 + GB].unsqueeze(2).broadcast_to([120, GB, H]),
                    Alu.is_equal,
                )
                bbig = bpool.tile([120, GB * H], fp32, tag="bbig")
                for j in range(GB * H // 512):
                    psb = ps_b.tile([120, 512], fp32, tag="psb")
                    nc.tensor.matmul(psb, lhsT=wgt, rhs=ohc[:, j * 512 : (j + 1) * 512], start=True, stop=True)
                    nc.vector.tensor_copy(bbig[:, j * 512 : (j + 1) * 512], psb)

                sg = stpool.tile([H, GB * H], fp32, tag="sg")
                for w in range(GB // 8):
                    psm = ps_map.tile([H, 512], fp32, tag="psm")
                    for s in range(8):
                        bl = w * 8 + s
                        nc.tensor.matmul(
                            psm[:, s * H : (s + 1) * H],
                            lhsT=rowsel[:, bl * H : (bl + 1) * H],
                            rhs=bbig[:, bl * H : (bl + 1) * H],
                            start=True,
                            stop=True,
                        )
                    nc.vector.tensor_copy(sg[:, w * 512 : (w + 1) * 512], psm)

                # flatten via DRAM roundtrip: contiguous spill of sg, then one
                # strided gather back so each batch map lands in one partition.
                d1 = dpool.tile([H, GB * H], fp32, tag="d1")
                nc.gpsimd.dma_start(d1[:, :], sg)
                Fg = fpool.tile([128, H * H], fp32, tag="F")
                nc.gpsimd.dma_start(
                    Fg[0 : 128 : 4, :].rearrange("b (y x) -> b y x", x=H),
                    d1[:, :].rearrange("y (b x) -> b y x", x=H),
                )

                # replicated output write: 24 channels per batch, 16KB runs
                # split each group's 24 channels across both HWDGE queues so
                # both queues stream from the first group onward
                HK = K // 2
                nc.sync.dma_start(
                    outT[b0 : b0 + GB, 0:HK],
                    Fg[0 : 128 : 4, :].unsqueeze(1).broadcast_to([GB, HK, H * H]),
                )
                nc.scalar.dma_start(
                    outT[b0 : b0 + GB, HK:K],
                    Fg[0 : 128 : 4, :].unsqueeze(1).broadcast_to([GB, HK, H * H]),
                )

    nc.compile()
    return nc


def _get_nc():
    if "nc" not in _CACHE:
        _CACHE["nc"] = _build_nc()
    return _CACHE["nc"]


def _host_inputs(x, weight, vis_batch, vis_kps):
    gflip = np.ascontiguousarray(weight[0, 0][::-1, ::-1]).astype(np.float32)
    wgm = np.zeros((120, 120), np.float32)
    idx = np.arange(K)
    for u in range(5):
        for c in range(5):
            wgm[c * K + idx, u * K + idx] = gflip[u, c]
    eye = np.eye(128, dtype=np.float32)
    vb = np.ascontiguousarray(vis_batch.astype(np.int32))
    vk = np.ascontiguousarray(vis_kps.astype(np.int32))
    in_maps = []
    for c in range(N_CORES):
        in_maps.append(
            {
                "xin": np.ascontiguousarray(
                    x[c * B_LOC : (c + 1) * B_LOC].reshape(B_LOC, K * 2).astype(np.float32)
                ),
                "visb": vb,
                "visk": vk,
                "boff": np.full((128, 1), c * B_LOC, np.float32),
                "wg": wgm,
                "eye": eye,
            }
        )
    return in_maps


def kernel(x, weight, vis_batch, vis_kps, _trace=False, _tmpdir=None):
    from concourse.bass_utils import run_bass_kernel_spmd

    nc = _get_nc()
    in_maps = _host_inputs(np.asarray(x), np.asarray(weight), np.asarray(vis_batch), np.asarray(vis_kps))
    res = run_bass_kernel_spmd(
        nc, in_maps, core_ids=list(range(N_CORES)), trace=_trace, tmpdir=_tmpdir
    )
    out = np.concatenate(
        [r["out"].reshape(B_LOC, K, H, H) for r in res.results], axis=0
    )
    if _trace:
        kernel._last_results = res
    return out

